# revision 5
# baseline (speedup 1.0000x reference)
"""Trainium2 Bass kernel for nn_BaselinePhasorBlock (B=2, L=1024, D=512, K=64).

Algorithm restructure: the phasor-memory cumsum
    retrieved[t,d] = Re[ sum_k e^{-i q[t,k]} * sum_{s<=t} e^{i key[s,k]} v[s,d] ]
collapses to causal attention:
    A[t,s] = cosQ[t]·cosK[s] + sinQ[t]·sinK[s]   (dot over k)
    retrieved = tril(A) @ value
so nothing of size (L,K,D) is ever materialized.

LayerNorm folding (exact):
    LN(retrieved/norm) @ Wo + bo + x
  = scale_t * (r @ Wg - mu_t * cw) + [x + ln_b@Wo + bo]
with Wg = diag(ln_g)@Wo, cw = colsums(Wg), scale_t = rsqrt(var_r + eps*norm_t^2),
norm_t^2 = (t+1)*K.

Sharding (8 cores, SPMD, no collectives): core c -> batch b = c//4, strip pair
i = c%4 owning t-strips [i*128,(i+1)*128) and [(7-i)*128,(8-i)*128).  Each core
computes its batch's keys and values over the full sequence (redundant across
the 4 cores of a batch, but avoids collectives).  The s-chunk order is
PERMUTED PER CORE so the core's own two strips are always local chunks 0,1:
query reads fixed addresses, causality is enforced by a data-driven mask
(tglob/sglob rows) built ON CHIP by DVE compares (no mask DMA).

v2 changes vs the 50.4us baseline (trace-driven):
  - DMAs split fine-grained in strict first-use order on the sync queue
    (wk1 halves + x quarters first) so the PE starts ~3us earlier; small /
    late tensors issue in parallel from the gpsimd queue.
  - gelu -> silu(1.702x)/1.702 (exact-table silu), putting silu+tanh+sin+abs
    +copy in ONE activation table set: 1 table load instead of 3, triggered
    by a dummy act at t=0.  The final rsqrt's table load is triggered by a
    dummy right after the Sins, off the critical tail.
  - mask (was 512KB DMA) built on chip; bv broadcast on chip (2KB DMA);
    xplus shipped bf16; output shipped bf16 (upcast on host).
  - scl = rsqrt(var + eps*n^2) as one ACT op (bias=epsn2).

All matmuls bf16, fp32 PSUM accumulation.
"""

import math
from contextlib import ExitStack

import numpy as np

B, L, D, K = 2, 1024, 512, 64
PI = math.pi
NCORES = 8
NSC = L // 128  # 8 s-chunks
NDC = D // 128  # 4 d-chunks
EPS = 1e-5
SILU_S = 1.702

# smalls_f32 column layout: [128, 20]
SM_BK1 = 0      # 4 cols  (1.702*bk1, packed (4,128).T)
SM_BQ1 = 4      # 4 cols
SM_BK2 = 8      # 1 col   (bk2 doubled, (128,1))
SM_BQ2 = 9      # 1 col
SM_EPSN = 10    # 2 cols  (eps*K*(tglob+1)) per strip
SM_SGLOB = 12   # 8 cols  (global s index of partition p in local chunk sc)
SM_W = 20

_CACHE = {}


def _build_program(gelu_override=None):
    import concourse.bacc as bacc
    import concourse.mybir as mybir
    import concourse.tile as tile

    AF = mybir.ActivationFunctionType
    ALU = mybir.AluOpType
    SILU = AF.Silu if gelu_override is None else gelu_override
    FP32 = mybir.dt.float32
    BF16 = mybir.dt.bfloat16

    nc = bacc.Bacc()

    # ---- DRAM tensors (DMA granularity = one tensor each) ----
    d_wk1a = nc.declare_dram_parameter("wk1a", [128, 1024], BF16, False)
    d_wk1b = nc.declare_dram_parameter("wk1b", [128, 1024], BF16, False)
    d_xq = [nc.declare_dram_parameter(f"xq{q}", [128, 1024], BF16, False)
            for q in range(4)]
    d_sm = nc.declare_dram_parameter("smalls", [128, SM_W], FP32, False)
    d_tg = nc.declare_dram_parameter("tg", [1, 256], FP32, False)
    d_wq1 = nc.declare_dram_parameter("wq1", [128, 2048], BF16, False)
    d_w2 = nc.declare_dram_parameter("w2", [128, 1024], BF16, False)
    d_wv = nc.declare_dram_parameter("wv", [128, 2048], BF16, False)
    d_rows = nc.declare_dram_parameter("rows", [1, 1024], BF16, False)
    d_wg = nc.declare_dram_parameter("wg", [128, 2048], BF16, False)
    d_xplus = nc.declare_dram_parameter("xplus", [128, 1024], BF16, False)
    d_out = nc.declare_dram_parameter("out", [2, 128, D], BF16, True)

    with tile.TileContext(nc) as tc, ExitStack() as ctx:
        consts = ctx.enter_context(tc.tile_pool(name="consts", bufs=1))
        work = ctx.enter_context(tc.tile_pool(name="work", bufs=1))
        atm_pool = ctx.enter_context(tc.tile_pool(name="atm", bufs=4))
        small = ctx.enter_context(tc.tile_pool(name="small", bufs=1))
        ps_big = ctx.enter_context(tc.tile_pool(name="ps_big", bufs=3, space="PSUM"))
        ps_at = ctx.enter_context(tc.tile_pool(name="ps_at", bufs=3, space="PSUM"))
        ps_rt = ctx.enter_context(tc.tile_pool(name="ps_rt", bufs=1, space="PSUM"))

        # ---- SBUF tiles ----
        wk1 = consts.tile([128, 2048], BF16)      # [c, j-cols] packed
        xt = consts.tile([128, 4096], BF16)       # quarters (m, cpair)
        sm = consts.tile([128, SM_W], FP32)
        tg = consts.tile([1, 256], FP32)
        wq1 = consts.tile([128, 2048], BF16)
        w2 = consts.tile([128, 1024], BF16)       # wk2d | wq2d
        wv = consts.tile([128, 2048], BF16)
        rows = consts.tile([1, 1024], BF16)       # cw | bv
        wg = consts.tile([128, 2048], BF16)
        xplus = consts.tile([128, 1024], BF16)

        ones = consts.tile([128, 1], BF16)
        junk = consts.tile([128, 1], FP32)
        cosbias = consts.tile([128, 1], FP32)
        sinscale = consts.tile([128, 1], FP32)
        tgb = consts.tile([128, 256], FP32)       # tglob bcast over partitions
        bvb = consts.tile([128, 512], BF16)       # bv bcast over partitions
        maskt = consts.tile([128, NSC, 256], BF16)

        wk1v = wk1.rearrange("p (c f) -> p c f", c=4)
        wq1v = wq1.rearrange("p (c f) -> p c f", c=4)
        w2k = w2[:, 0:512].rearrange("p (c f) -> p c f", c=4)
        w2q = w2[:, 512:1024].rearrange("p (c f) -> p c f", c=4)
        wvv = wv.rearrange("p (c f) -> p c f", c=4)
        wgv = wg.rearrange("p (c f) -> p c f", c=4)
        xplusv = xplus.rearrange("p (s f) -> p s f", s=2)
        cw = rows[:, 0:512]
        bvrow = rows[:, 512:1024]

        def xT(c, lo, hi):
            """x^T slice [din-chunk c, local seq cols lo:hi]; lo//512==(hi-1)//512."""
            m = lo // 512
            q = m * 2 + c // 2
            base = q * 1024 + (c % 2) * 512
            return xt[:, base + lo - m * 512: base + hi - m * 512]

        # ---- DMAs: critical stream on sync (strict first-use order);
        #      small/late tensors in parallel on gpsimd ----
        nc.sync.dma_start(out=wk1[:, 0:1024], in_=d_wk1a[:])
        nc.sync.dma_start(out=xt[:, 0:1024], in_=d_xq[0][:])
        nc.sync.dma_start(out=wk1[:, 1024:2048], in_=d_wk1b[:])
        nc.sync.dma_start(out=xt[:, 1024:2048], in_=d_xq[1][:])
        nc.sync.dma_start(out=xt[:, 2048:3072], in_=d_xq[2][:])
        nc.sync.dma_start(out=xt[:, 3072:4096], in_=d_xq[3][:])
        nc.sync.dma_start(out=wq1, in_=d_wq1[:])
        nc.sync.dma_start(out=w2, in_=d_w2[:])
        nc.sync.dma_start(out=wv, in_=d_wv[:])
        nc.gpsimd.dma_start(out=sm, in_=d_sm[:])
        nc.gpsimd.dma_start(out=tg, in_=d_tg[:])
        nc.gpsimd.dma_start(out=rows, in_=d_rows[:])
        nc.gpsimd.dma_start(out=wg, in_=d_wg[:])
        nc.gpsimd.dma_start(out=xplus, in_=d_xplus[:])

        # ---- preamble constants + act-table preload (all hidden) ----
        nc.vector.memset(ones, 1.0)
        nc.vector.memset(junk, 1.0)
        nc.vector.memset(cosbias[0:64, :], PI / 2)
        nc.vector.memset(cosbias[64:128, :], 0.0)
        nc.vector.memset(sinscale[0:64, :], -PI)
        nc.vector.memset(sinscale[64:128, :], PI)
        # dummy: forces the silu/tanh/sin/abs table load at t~0
        nc.scalar.activation(out=junk, in_=ones, func=SILU)

        # on-chip broadcasts + causal mask build (DVE/Pool, off critical path)
        nc.gpsimd.partition_broadcast(tgb, tg[:])
        nc.gpsimd.partition_broadcast(bvb, bvrow)
        for sc in range(NSC):
            nc.vector.tensor_scalar(
                out=maskt[:, sc, :], in0=tgb,
                scalar1=sm[:, SM_SGLOB + sc:SM_SGLOB + sc + 1], scalar2=None,
                op0=ALU.is_ge,
            )

        # ---- working tiles ----
        hkT = work.tile([128, 4, L], BF16)
        hqT = work.tile([128, 4, 256], BF16)
        kph2 = work.tile([128, L], BF16)
        qph2 = work.tile([128, 256], BF16)
        KS = work.tile([128, L], BF16)          # rows 0:64 cos, 64:128 sin
        QS = work.tile([128, 256], BF16)
        value = work.tile([128, NSC, D], BF16)
        rT_sb = work.tile([128, 4, 256], BF16)
        rsq = work.tile([128, 4, 256], BF16)
        out_sb = work.tile([128, 2, D], BF16)

        # ---- MLP1 (key): hkT[j, l] = silu(1.702*(Wk1^T @ xT + bk1)) ----
        for m in range(2):
            for j in range(4):
                ps = ps_big.tile([128, 512], FP32, tag="mlp")
                for c in range(4):
                    nc.tensor.matmul(
                        ps,
                        lhsT=wk1v[:, c, j * 128:(j + 1) * 128],
                        rhs=xT(c, m * 512, (m + 1) * 512),
                        start=(c == 0),
                        stop=(c == 3),
                    )
                nc.scalar.activation(
                    out=hkT[:, j, m * 512:(m + 1) * 512], in_=ps,
                    func=SILU, bias=sm[:, SM_BK1 + j:SM_BK1 + j + 1],
                    scale=SILU_S,
                )

        # ---- MLP1 (query): own strips are local chunks 0,1 -> xT(c, 0:256) ----
        for j in range(4):
            ps = ps_big.tile([128, 512], FP32, tag="mlp")
            for c in range(4):
                nc.tensor.matmul(
                    ps[:, 0:256],
                    lhsT=wq1v[:, c, j * 128:(j + 1) * 128],
                    rhs=xT(c, 0, 256),
                    start=(c == 0),
                    stop=(c == 3),
                )
            nc.scalar.activation(
                out=hqT[:, j, :], in_=ps[:, 0:256],
                func=SILU, bias=sm[:, SM_BQ1 + j:SM_BQ1 + j + 1],
                scale=SILU_S,
            )

        # ---- phase matmuls + tanh (duplicated halves via doubled W2) ----
        for m in range(2):
            ps_k = ps_big.tile([128, 512], FP32, tag="mlp")
            for j in range(4):
                nc.tensor.matmul(
                    ps_k,
                    lhsT=w2k[:, j, :],
                    rhs=hkT[:, j, m * 512:(m + 1) * 512],
                    start=(j == 0),
                    stop=(j == 3),
                )
            nc.scalar.activation(out=kph2[:, m * 512:(m + 1) * 512],
                                 in_=ps_k, func=AF.Tanh,
                                 bias=sm[:, SM_BK2:SM_BK2 + 1], scale=1.0)
        ps_p = ps_big.tile([128, 512], FP32, tag="mlp")
        for j in range(4):
            nc.tensor.matmul(
                ps_p[:, 0:256],
                lhsT=w2q[:, j, :],
                rhs=hqT[:, j, :],
                start=(j == 0),
                stop=(j == 3),
            )
        nc.scalar.activation(out=qph2, in_=ps_p[:, 0:256], func=AF.Tanh,
                             bias=sm[:, SM_BQ2:SM_BQ2 + 1], scale=1.0)

        # ---- |t| on the cos half; then one Sin pass with per-partition
        #      scale/bias gives stacked cos/sin ----
        nc.scalar.activation(out=kph2[0:64, :], in_=kph2[0:64, :], func=AF.Abs)
        nc.scalar.activation(out=qph2[0:64, :], in_=qph2[0:64, :], func=AF.Abs)
        nc.scalar.activation(out=KS, in_=kph2, func=AF.Sin,
                             bias=cosbias, scale=sinscale)
        nc.scalar.activation(out=QS, in_=qph2, func=AF.Sin,
                             bias=cosbias, scale=sinscale)
        # dummy: trigger the sqrt table load now (ACT idle, tail unblocked)
        nc.scalar.activation(out=junk, in_=junk, func=AF.Sqrt)

        # ---- value rows ----
        for sc in range(NSC):
            ps = ps_at.tile([128, 512], FP32, tag="at")
            for c in range(4):
                nc.tensor.matmul(
                    ps,
                    lhsT=xT(c, sc * 128, (sc + 1) * 128),
                    rhs=wvv[:, c, :],
                    start=(c == 0),
                    stop=(c == 3),
                )
            nc.vector.tensor_add(out=value[:, sc, :], in0=ps, in1=bvb)

        # ---- scores + causal mask + retrievedT accumulation ----
        rt_ps = ps_rt.tile([128, 4, 256], FP32)
        for sc in range(NSC):
            at_ps = ps_at.tile([128, 256], FP32, tag="at")
            nc.tensor.matmul(
                at_ps,
                lhsT=KS[:, sc * 128:(sc + 1) * 128],
                rhs=QS,
                start=True,
                stop=True,
            )
            atm = atm_pool.tile([128, 256], BF16, tag="atm")
            nc.vector.tensor_mul(out=atm, in0=at_ps, in1=maskt[:, sc, :])
            for dc in range(NDC):
                nc.tensor.matmul(
                    rt_ps[:, dc, :],
                    lhsT=value[:, sc, dc * 128:(dc + 1) * 128],
                    rhs=atm,
                    start=(sc == 0 and dc in (0, 2)),
                    stop=(sc == NSC - 1 and dc in (1, 3)),
                )

        # ---- retrievedT -> SBUF + squares ----
        for dc in range(NDC):
            nc.scalar.copy(out=rT_sb[:, dc, :], in_=rt_ps[:, dc, :])
        for dc in range(NDC):
            nc.vector.tensor_mul(out=rsq[:, dc, :], in0=rT_sb[:, dc, :],
                                 in1=rT_sb[:, dc, :])

        # ---- row stats ----
        sums_ps = ps_at.tile([128, 4], FP32, tag="at")
        row_ps = ps_at.tile([1, 256], FP32, tag="at")
        first = True
        n = 0
        for st in range(2):
            for src, col in ((rT_sb, st), (rsq, 2 + st)):
                for dc in range(NDC):
                    n += 1
                    nc.tensor.matmul(
                        sums_ps[:, col:col + 1],
                        lhsT=src[:, dc, st * 128:(st + 1) * 128],
                        rhs=ones,
                        start=first,
                        stop=(n == 16),
                    )
                    first = False
        for dc in range(NDC):
            nc.tensor.matmul(
                row_ps,
                lhsT=ones,
                rhs=rT_sb[:, dc, :],
                start=(dc == 0),
                stop=(dc == 3),
            )

        negmu = small.tile([1, 256], BF16)
        nc.vector.tensor_scalar_mul(out=negmu, in0=row_ps, scalar1=-1.0 / D)

        mu = small.tile([128, 2], FP32)
        musq = small.tile([128, 2], FP32)
        var = small.tile([128, 2], FP32)
        scl = small.tile([128, 2], FP32)
        for st in range(2):
            nc.vector.tensor_scalar_mul(out=mu[:, st:st + 1],
                                        in0=sums_ps[:, st:st + 1],
                                        scalar1=1.0 / D)
            nc.vector.tensor_mul(out=musq[:, st:st + 1],
                                 in0=mu[:, st:st + 1], in1=mu[:, st:st + 1])
            nc.vector.scalar_tensor_tensor(
                out=var[:, st:st + 1],
                in0=sums_ps[:, 2 + st:3 + st],
                scalar=1.0 / D,
                in1=musq[:, st:st + 1],
                op0=ALU.mult,
                op1=ALU.subtract,
            )
        for st in range(2):
            nc.scalar.activation(out=scl[:, st:st + 1], in_=var[:, st:st + 1],
                                 func=AF.Sqrt,
                                 bias=sm[:, SM_EPSN + st:SM_EPSN + st + 1],
                                 scale=1.0)
        nc.vector.reciprocal(out=scl, in_=scl)

        # ---- output: out = scale * (rT^T @ Wg - mu*cw) + xplus ----
        for st in range(2):
            ps = ps_big.tile([128, 512], FP32, tag="mlp")
            for dc in range(NDC):
                nc.tensor.matmul(
                    ps,
                    lhsT=rT_sb[:, dc, st * 128:(st + 1) * 128],
                    rhs=wgv[:, dc, :],
                    start=(dc == 0),
                    stop=False,
                )
            nc.tensor.matmul(
                ps,
                lhsT=negmu[:, st * 128:(st + 1) * 128],
                rhs=cw,
                start=False,
                stop=True,
            )
            nc.vector.scalar_tensor_tensor(
                out=out_sb[:, st, :],
                in0=ps,
                scalar=scl[:, st:st + 1],
                in1=xplusv[:, st, :],
                op0=ALU.mult,
                op1=ALU.add,
            )
            nc.sync.dma_start(out=d_out[st], in_=out_sb[:, st, :])

    return nc


def _host_prepare(inputs):
    """Build the 8 per-core input maps (host-side numpy packing)."""
    import ml_dtypes

    bf16 = ml_dtypes.bfloat16
    f32 = np.float32

    x = np.asarray(inputs["x"], f32)
    Wk1 = np.asarray(inputs["Wk1"], f32)
    bk1 = np.asarray(inputs["bk1"], f32)
    Wk2 = np.asarray(inputs["Wk2"], f32)
    bk2 = np.asarray(inputs["bk2"], f32)
    Wq1 = np.asarray(inputs["Wq1"], f32)
    bq1 = np.asarray(inputs["bq1"], f32)
    Wq2 = np.asarray(inputs["Wq2"], f32)
    bq2 = np.asarray(inputs["bq2"], f32)
    Wv = np.asarray(inputs["Wv"], f32)
    bv = np.asarray(inputs["bv"], f32)
    ln_g = np.asarray(inputs["ln_g"], f32)
    ln_b = np.asarray(inputs["ln_b"], f32)
    Wo = np.asarray(inputs["Wo"], f32)
    bo = np.asarray(inputs["bo"], f32)

    Wg32 = ln_g[:, None] * Wo
    cw = Wg32.astype(bf16).astype(f32).sum(axis=0).astype(bf16).reshape(1, D)
    out_bias = (ln_b @ Wo + bo).astype(f32)

    def pack(w):  # [D_in, F] -> [128, 4, F] -> flat [128, 4*F]
        return w.reshape(4, 128, -1).transpose(1, 0, 2)

    wk1p = pack(Wk1).reshape(128, 2048).astype(bf16)
    wq1p = pack(Wq1).reshape(128, 2048).astype(bf16)
    wvp = pack(Wv).reshape(128, 2048).astype(bf16)
    wgp = pack(Wg32).reshape(128, 2048).astype(bf16)
    wk2d = pack(np.concatenate([Wk2, Wk2], axis=1) / SILU_S).reshape(128, 512)
    wq2d = pack(np.concatenate([Wq2, Wq2], axis=1) / SILU_S).reshape(128, 512)
    w2p = np.concatenate([wk2d, wq2d], axis=1).astype(bf16)

    sm_base = np.zeros((128, SM_W), f32)
    sm_base[:, SM_BK1:SM_BK1 + 4] = SILU_S * bk1.reshape(4, 128).T
    sm_base[:, SM_BQ1:SM_BQ1 + 4] = SILU_S * bq1.reshape(4, 128).T
    sm_base[:, SM_BK2] = np.concatenate([bk2, bk2])
    sm_base[:, SM_BQ2] = np.concatenate([bq2, bq2])

    rows_arr = np.zeros((1, 1024), bf16)
    rows_arr[0, 0:512] = cw[0]
    rows_arr[0, 512:1024] = bv.astype(bf16)

    in_maps = []
    for c in range(NCORES):
        b, i = divmod(c, 4)
        # local s-chunk order: own strips first
        sigma = [i, 7 - i] + [s for s in range(8) if s not in (i, 7 - i)]
        xb = x[b]  # [L, D]
        perm = np.concatenate([np.arange(s * 128, (s + 1) * 128) for s in sigma])
        xp = xb[perm]                      # [L, D] permuted rows
        xTp = pack(np.ascontiguousarray(xp.T))  # [128, 4, 1024] (c, l)
        # quarters: q = m*2 + cpair, cols = [c_in_pair 0 | 1] within (m)
        xq = []
        for m in range(2):
            for cp in range(2):
                qa = np.concatenate(
                    [xTp[:, 2 * cp, m * 512:(m + 1) * 512],
                     xTp[:, 2 * cp + 1, m * 512:(m + 1) * 512]], axis=1)
                xq.append(qa.astype(bf16))

        tglob = np.concatenate(
            [np.arange(i * 128, (i + 1) * 128),
             np.arange((7 - i) * 128, (8 - i) * 128)]).astype(f32)
        sglob = (np.array(sigma, dtype=f32)[None, :] * 128.0
                 + np.arange(128, dtype=f32)[:, None])  # [128, 8]

        sm = sm_base.copy()
        sm[:, SM_EPSN:SM_EPSN + 2] = (EPS * K * (tglob + 1.0)).reshape(2, 128).T
        sm[:, SM_SGLOB:SM_SGLOB + 8] = sglob

        xplus = (np.stack([xb[i * 128:(i + 1) * 128],
                           xb[(7 - i) * 128:(8 - i) * 128]]) + out_bias)
        xplus = xplus.transpose(1, 0, 2).reshape(128, 1024).astype(bf16)

        m = {
            "wk1a": wk1p[:, 0:1024], "wk1b": wk1p[:, 1024:2048],
            "xq0": xq[0], "xq1": xq[1], "xq2": xq[2], "xq3": xq[3],
            "smalls": sm,
            "tg": tglob.reshape(1, 256),
            "wq1": wq1p, "w2": w2p, "wv": wvp,
            "rows": rows_arr, "wg": wgp, "xplus": xplus,
        }
        in_maps.append(m)
    return in_maps


def run(inputs, trace=False):
    from concourse.bass_utils import run_bass_kernel_spmd

    if "nc" not in _CACHE:
        nc = _build_program()
        nc.finalize()
        _CACHE["nc"] = nc
    nc = _CACHE["nc"]
    in_maps = _host_prepare(inputs)
    res = run_bass_kernel_spmd(nc, in_maps, list(range(NCORES)), trace=trace)
    out = np.empty((B, L, D), np.float32)
    for c in range(NCORES):
        b, i = divmod(c, 4)
        oc = np.asarray(res.results[c]["out"]).astype(np.float32)
        out[b, i * 128:(i + 1) * 128] = oc[0]
        out[b, (7 - i) * 128:(8 - i) * 128] = oc[1]
    return out, res


def kernel(**inputs):
    out, _ = run(inputs, trace=False)
    return out
